# revision 7
# baseline (speedup 1.0000x reference)
"""3D Canny edge detector on 8 Trainium2 cores.

Shard D=256 across 8 cores (32 output slices each) with a 4-voxel halo,
entirely host-side (no collectives). Per-core layout: partitions =
3 h-strips x 40 local d-slices (120 of 128), free dim = (94 h-rows, 52 w-cols)
per w-tile. All three stencil axes are then partition- or free-dim shifts.
sqrt is eliminated by comparing squared magnitudes against squared thresholds;
the Gaussian is applied unnormalized ([u,1,u] per axis) with the normalization
folded into the thresholds. Global-border zeroing of the gradient magnitude is
done via a per-partition mask input (d borders, differs per core) fused into
the ScalarE square ops, plus tiny memsets for the h/w border rows/cols.
"""
import json
import numpy as np

import concourse.bass as bass
import concourse.mybir as mybir
from concourse.bass_utils import run_bass_kernel_spmd
from concourse.tile import TileContext

F32 = mybir.dt.float32
I8 = mybir.dt.int8
AL = mybir.AluOpType
SQ = mybir.ActivationFunctionType.Square

N_CORES = 8
D, H, W = 256, 256, 256
DLOC = 40           # 32 output slices + 4 halo each side
NPART = 120         # 3 strips * 40
ROWS = 94           # h rows per strip tile (out rows + up to 4 halo each side)
COLS = 52           # w cols per tile (44 out + 4 halo each side)
WT_OUT = 44
N_WT = 6
STRIP_OFF = (0, 85, 170)                       # padded-h offset per strip
STRIP_OUT = ((4, 86, 0), (5, 85, 86), (5, 85, 171))  # (first r, n rows, h0)

U = float(np.exp(np.float64(-0.5)))
SC = (1.0 + 2.0 * U) ** 3
HI2 = float((0.2 * SC) ** 2)
LO2 = float((0.1 * SC) ** 2)


def _fix_bir_json_bytes(raw: bytes) -> bytes:
    """walrus codegen has per-instruction sync-wait-slot limits (1 for CTRL
    Drain, 2 for compute structs). Hoist excess waits onto prepended
    single-wait Drain instructions on the same engine."""
    m = json.loads(raw)
    changed = False
    for fn in m.get("functions", []):
        for bb in fn.get("blocks", []):
            out = []
            for inst in bb.get("instructions", []):
                si = inst.get("sync_info") or {}
                waits = si.get("on_wait") or []
                lim = 1
                if len(waits) > lim and inst.get("engine") not in (None, "Unassigned"):
                    changed = True
                    keep_n = lim
                    for i, wt in enumerate(waits[:-keep_n] if keep_n else waits):
                        out.append({
                            "debug": inst.get("debug", 0),
                            "engine": inst["engine"],
                            "ins": [], "outs": [],
                            "is_reset_sema": False,
                            "name": f"{inst['name']}-w{i}",
                            "opcode": "Drain",
                            "sync_info": {"on_update": [], "on_wait": [wt]},
                        })
                    si["on_wait"] = waits[-keep_n:] if keep_n else []
                    inst["sync_info"] = si
                out.append(inst)
            bb["instructions"] = out
    return json.dumps(m).encode() if changed else raw


def _build():
    nc = bass.Bass("TRN2", target_bir_lowering=False, debug=False, num_devices=1)
    x = nc.dram_tensor("x", [DLOC, 264, 264], F32, kind="ExternalInput").ap()
    dmask = nc.dram_tensor("dmask", [NPART, 1], F32, kind="ExternalInput").ap()
    y = nc.dram_tensor("y", [32, H, W], I8, kind="ExternalOutput").ap()

    _n = [0]

    def _ctr():
        _n[0] += 1
        return _n[0]

    with TileContext(nc) as tc:
        with tc.tile_pool(name="p", bufs=1) as pool:
            dm = pool.tile([NPART, 1], F32, tag="dm", name="dm0")
            nc.gpsimd.dma_start(out=dm[:], in_=dmask[:])
            zrow = pool.tile([NPART, COLS], F32, tag="zr", name="zr0")
            nc.gpsimd.memset(zrow[:], 0.0)

            for t in range(N_WT):
                c0 = WT_OUT * t
                in_w = min(COLS, 264 - c0)

                def T(tag):
                    return pool.tile([NPART, ROWS, COLS], F32, tag=tag, name=f"{tag}_{t}_{_ctr()}")

                v = nc.vector
                xt = T("S1")
                for s in range(3):
                    nc.gpsimd.dma_start(
                        out=xt[s * DLOC:(s + 1) * DLOC, :, 0:in_w],
                        in_=x[:, STRIP_OFF[s]:STRIP_OFF[s] + ROWS, c0:c0 + in_w],
                    )
                # ---- Gaussian [u,1,u] along w, h, d ----
                tw = T("S2")
                v.tensor_tensor(tw[:, :, 1:51], xt[:, :, 0:50], xt[:, :, 2:52], AL.add)
                smw = T("S3")
                v.scalar_tensor_tensor(smw[:, :, 1:51], tw[:, :, 1:51], U,
                                       xt[:, :, 1:51], AL.mult, AL.add)
                th = T("S2")
                v.tensor_tensor(th[:, 1:93, :], smw[:, 0:92, :], smw[:, 2:94, :], AL.add)
                smwh = T("S1")
                v.scalar_tensor_tensor(smwh[:, 1:93, :], th[:, 1:93, :], U,
                                       smw[:, 1:93, :], AL.mult, AL.add)
                # d-shift staging copies (DMA partition realign; compute stays
                # at partition start 0 per ISA 32-alignment rule)
                sp = T("S7")
                nc.gpsimd.dma_start(out=sp[0:119], in_=smwh[1:120])
                sn = T("S8")
                nc.gpsimd.dma_start(out=sn[1:120], in_=smwh[0:119])
                td = T("S2")
                v.tensor_tensor(td[:], sn[:], sp[:], AL.add)
                sm = T("S3")
                v.scalar_tensor_tensor(sm[:], td[:], U, smwh[:], AL.mult, AL.add)
                # ---- Sobel d-stage: A = sm*[1,1,1]_d, B = sm*[-1,0,1]_d ----
                p2 = T("S7")
                nc.gpsimd.dma_start(out=p2[0:119], in_=sm[1:120])
                m2 = T("S8")
                nc.gpsimd.dma_start(out=m2[1:120], in_=sm[0:119])
                a1 = T("S2")
                v.tensor_tensor(a1[:], p2[:], m2[:], AL.add)
                A = T("S1")
                v.tensor_tensor(A[:], a1[:], sm[:], AL.add)
                B = T("S2")
                v.tensor_tensor(B[:], p2[:], m2[:], AL.subtract)
                # ---- gx = A *h [1,2,1] *w [-1,0,1] ----
                ph = T("S3")
                v.tensor_tensor(ph[:, 2:92, :], A[:, 1:91, :], A[:, 3:93, :], AL.add)
                gxh = T("S4")
                v.scalar_tensor_tensor(gxh[:, 2:92, :], A[:, 2:92, :], 2.0,
                                       ph[:, 2:92, :], AL.mult, AL.add)
                gx = T("S3")
                v.tensor_tensor(gx[:, :, 2:50], gxh[:, :, 3:51], gxh[:, :, 1:49],
                                AL.subtract)
                # ---- gy = A *h [-1,0,1] *w [1,2,1] ----
                gyh = T("S4")
                v.tensor_tensor(gyh[:, 2:92, :], A[:, 3:93, :], A[:, 1:91, :],
                                AL.subtract)
                pw = T("S5")
                v.tensor_tensor(pw[:, :, 2:50], gyh[:, :, 1:49], gyh[:, :, 3:51], AL.add)
                gy = T("S6")
                v.scalar_tensor_tensor(gy[:, :, 2:50], gyh[:, :, 2:50], 2.0,
                                       pw[:, :, 2:50], AL.mult, AL.add)
                # ---- gz = B *h [1,1,1] *w [1,1,1] ----
                bh1 = T("S1")
                v.tensor_tensor(bh1[:, 2:92, :], B[:, 1:91, :], B[:, 3:93, :], AL.add)
                bh = T("S4")
                v.tensor_tensor(bh[:, 2:92, :], bh1[:, 2:92, :], B[:, 2:92, :], AL.add)
                bw1 = T("S1")
                v.tensor_tensor(bw1[:, :, 2:50], bh[:, :, 1:49], bh[:, :, 3:51], AL.add)
                gz = T("S2")
                v.tensor_tensor(gz[:, :, 2:50], bw1[:, :, 2:50], bh[:, :, 2:50], AL.add)
                # ---- msq = dmask*(gx^2+gy^2+gz^2), then h/w border zeroing ----
                sx = T("S1")
                nc.scalar.activation(sx[:], gx[:], SQ, scale=dm[:, 0:1])
                sy = T("S4")
                nc.scalar.activation(sy[:], gy[:], SQ, scale=dm[:, 0:1])
                sz = T("S6")
                nc.scalar.activation(sz[:], gz[:], SQ, scale=dm[:, 0:1])
                m1 = T("S2")
                v.tensor_tensor(m1[:], sx[:], sy[:], AL.add)
                msq = T("S1")
                v.tensor_tensor(msq[:], m1[:], sz[:], AL.add)
                nc.gpsimd.dma_start(out=msq[0:40, 4:5, :], in_=zrow[0:40, :])
                nc.gpsimd.dma_start(out=msq[80:120, 89:90, :], in_=zrow[80:120, :])
                if t == 0:
                    nc.gpsimd.memset(msq[:, :, 4:5], 0.0)
                if t == N_WT - 1:
                    nc.gpsimd.memset(msq[:, :, 39:40], 0.0)
                # ---- NMS ----
                r2 = T("S2")
                v.tensor_tensor(r2[:, :, 3:49], msq[:, :, 2:48], msq[:, :, 4:50], AL.max)
                r3 = T("S3")
                v.tensor_tensor(r3[:, :, 3:49], r2[:, :, 3:49], msq[:, :, 3:49], AL.max)
                mh = T("S4")
                v.tensor_tensor(mh[:, 3:91, :], r3[:, 2:90, :], r3[:, 4:92, :], AL.max)
                nb8 = T("S3")
                v.tensor_tensor(nb8[:, 3:91, :], mh[:, 3:91, :], r2[:, 3:91, :], AL.max)
                nbm = T("S7")
                nc.gpsimd.dma_start(out=nbm[1:120], in_=nb8[0:119])
                keep = T("S2")
                v.tensor_tensor(keep[:], msq[:], nbm[:], AL.is_gt)
                nmsq = T("S3")
                v.tensor_tensor(nmsq[:], msq[:], keep[:], AL.mult)
                # ---- thresholds ----
                strong = T("S1")
                v.tensor_scalar(strong[:], nmsq[:], HI2, None, AL.is_gt)
                weakish = T("S2")
                v.tensor_scalar(weakish[:], nmsq[:], LO2, None, AL.is_gt)
                weak = T("S3")
                v.tensor_tensor(weak[:], weakish[:], strong[:], AL.subtract)
                # ---- hysteresis ----
                tp = T("S7")
                nc.gpsimd.dma_start(out=tp[0:119], in_=strong[1:120])
                tm = T("S8")
                nc.gpsimd.dma_start(out=tm[1:120], in_=strong[0:119])
                sd = T("S2")
                v.tensor_tensor(sd[:], tp[:], tm[:], AL.add)
                sh = T("S4")
                v.tensor_tensor(sh[:, 4:90, :], strong[:, 3:89, :], strong[:, 5:91, :],
                                AL.add)
                sw = T("S5")
                v.tensor_tensor(sw[:, :, 4:48], strong[:, :, 3:47], strong[:, :, 5:49],
                                AL.add)
                sa = T("S6")
                v.tensor_tensor(sa[:], sd[:], sh[:], AL.add)
                any6 = T("S2")
                v.tensor_tensor(any6[:], sa[:], sw[:], AL.add)
                wa = T("S4")
                v.scalar_tensor_tensor(wa[:], any6[:], 0.5, weak[:], AL.is_ge, AL.mult)
                out01 = pool.tile([NPART, ROWS, COLS], I8, tag="o8", name=f"o8_{t}")
                v.tensor_tensor(out01[:], wa[:], strong[:], AL.max)

                ow = WT_OUT if t < N_WT - 1 else 36
                for s in range(3):
                    r0, nr, h0 = STRIP_OUT[s]
                    nc.gpsimd.dma_start(
                        out=y[:, h0:h0 + nr, WT_OUT * t:WT_OUT * t + ow],
                        in_=out01[s * DLOC + 4:s * DLOC + 36, r0:r0 + nr, 4:4 + ow],
                    )
    orig = nc.to_json_bytes
    nc.to_json_bytes = lambda: _fix_bir_json_bytes(orig())
    return nc


_NC_CACHE = None


def kernel(x: np.ndarray) -> np.ndarray:
    global _NC_CACHE
    x3 = np.ascontiguousarray(x[0], dtype=np.float32)
    xp = np.pad(x3, 1, mode="reflect")                # (258,258,258)
    xp = np.pad(xp, ((0, 0), (3, 3), (3, 3)))         # (258,264,264)

    in_maps = []
    for c in range(N_CORES):
        g0 = 32 * c
        slab = np.zeros((DLOC, 264, 264), np.float32)
        lo = max(0, g0 - 3)            # xp d-index = global+1, want [g0-3, g0+37)
        hi = min(258, g0 + 37)
        slab[lo - (g0 - 3):hi - (g0 - 3)] = xp[lo:hi]
        dmv = np.ones((NPART, 1), np.float32)
        if c == 0:
            dmv[[4, 44, 84]] = 0.0
        if c == N_CORES - 1:
            dmv[[35, 75, 115]] = 0.0
        in_maps.append({"x": slab, "dmask": dmv})

    if _NC_CACHE is None:
        _NC_CACHE = _build()
    res = run_bass_kernel_spmd(_NC_CACHE, in_maps, list(range(N_CORES)))
    out = np.concatenate([r["y"] for r in res.results], axis=0)
    return out[None].astype(np.int8)
